# revision 85
# baseline (speedup 1.0000x reference)
"""Trainium2 Bass kernel for nn_BatchHighOrderActivation.

Math: out[b,i,o] = sum_k coef_k * params[i, idx_k, o]  (sorted-diff coefs,
reverse-cumsum subset masks).  Rewritten gather-free as

    out[b,i,:] = sum_{m=1..15} w_m[b,i] * params[i, m, :]
    w_m = relu( min_{j in m} X_j  -  max_{j not in m} X_j )   (m != 15)
    w_15 = min4 = relu(min4) - relu(-min4)  (split across two relu slots)

Per core (batch-sharded 8 ways, 1024 batch rows each), per 128-row b-tile:
  1. SP HWDGE loads X tile (host pre-casts X to bf16).
  2. Pool: deinterleave -> 4 planes X_j (strided tensor_copy).
  3. DVE: the full min/max lattice (pmin/pmax/tmin/tmax/s14) -- GPSIMD has
     no tensor-tensor min/max ucode, so these must stay on DVE (2x bf16).
  4. Pool: the 14 slot subtractions (GPSIMD supports subtract; all operand
     access patterns ascending -- GPSIMD also rejects negative-stride and
     broadcast APs) + the s15 negation.
  5. PE:  transpose W group-tiles ([128b x 128q]) -> PSUM bf16.
  6. ACT/DVE: relu-evacuate PSUM -> lhsT tiles [128q, 128b] bf16.
     (GPSIMD cannot touch PSUM.)
  7. PE:  matmul lhsT.T @ PD[g] (PD = block-diag P, K=q) -> PSUM fp32.
  8. ACT: cast-copy PSUM fp32 -> SBUF bf16.
  9. SP:  DMA out bf16 (host upcasts to fp32 after gather).

Engine busy budget per core (v1 cost model): DVE ~136us (lattice + relu
share), ACT ~136us (relu share + out-evacs), Pool ~130us (deint + subs),
SP ~82us DMA, PE ~82us.
"""

import sys

for _p in ("/opt/trn_rl_repo", "/root/.axon_site/_ro/trn_rl_repo"):
    if _p not in sys.path:
        sys.path.append(_p)

import numpy as np
import ml_dtypes

B, I, A, O = 8192, 1024, 4, 8
NCORES = 8
BC = B // NCORES          # batch rows per core
NG = I // 8               # 128 groups of 8 i-rows
NSLOT = 16

# slot order chosen so merged double-width subs write adjacent slots:
# s0..3 singles {0}{1}{2}{3}; s4..9 pair-masks in PAIRS order; s10..13
# triples ordered by excluded coordinate; s14/15 = +/- full-set (mask 15)
SLOT_MASKS = [1, 2, 4, 8, 3, 12, 5, 10, 9, 6, 14, 13, 11, 7]

_CACHE = {}


def _build_pd(params: np.ndarray) -> np.ndarray:
    """Block-diagonal P table: PD[q = s*8 + i_sub, g, n = i_sub*8 + o]."""
    Pt = np.empty((I, NSLOT, O), np.float32)
    for s, m in enumerate(SLOT_MASKS):
        Pt[:, s, :] = params[:, m, :]
    Pt[:, 14, :] = params[:, 15, :]
    Pt[:, 15, :] = -params[:, 15, :]

    PD = np.zeros((128, NG, 64), np.float32)
    for s in range(NSLOT):
        for isub in range(8):
            PD[s * 8 + isub, :, isub * 8:(isub + 1) * 8] = Pt[
                np.arange(NG) * 8 + isub, s, :
            ]
    return PD.reshape(128, NG * 64).astype(ml_dtypes.bfloat16)


def _build_bass():
    import concourse.bass as bass
    import concourse.mybir as mybir
    import concourse.tile as tile
    from concourse import bacc
    from concourse.masks import make_identity

    f32 = mybir.dt.float32
    wdt = mybir.dt.bfloat16

    nc = bacc.Bacc(None)
    Xp = nc.declare_dram_parameter("X", [BC, I, A], wdt, isOutput=False)
    PDp = nc.declare_dram_parameter("PD", [128, NG * 64], wdt, isOutput=False)
    OUTp = nc.declare_dram_parameter("OUT", [BC, I, O], wdt, isOutput=True)

    AF = mybir.ActivationFunctionType
    ALU = mybir.AluOpType

    IH = I // 2    # i-half extent per lattice pass

    with tile.TileContext(nc) as tc:
        with (
            tc.tile_pool(name="consts", bufs=1) as consts,
            tc.tile_pool(name="xin", bufs=5) as xin_pool,
            tc.tile_pool(name="xj", bufs=3) as xj_pool,
            tc.tile_pool(name="scr", bufs=2) as scr_pool,
            tc.tile_pool(name="w", bufs=3) as w_pool,
            tc.tile_pool(name="lh", bufs=4) as lh_pool,
            tc.tile_pool(name="ot", bufs=5) as ot_pool,
            tc.tile_pool(name="psT", bufs=2, space="PSUM") as psT_pool,
            tc.tile_pool(name="psO", bufs=2, space="PSUM") as psO_pool,
        ):
            ident = consts.tile([128, 128], wdt)
            make_identity(nc, ident)
            pd_sb = consts.tile([128, NG * 64], wdt)

            NT = BC // 128
            # X tiles prefetched with lookahead >= 2 so the loads run ahead
            # of the same-tile OUT DMAs in the SP queue's program order
            xts = {}

            def load_x(tt):
                if tt >= NT or tt in xts:
                    return
                bs = slice(tt * 128, (tt + 1) * 128)
                xt = xin_pool.tile([128, I, A], wdt)
                # tile 0 loads in quarters so the first 256-row chunk's
                # deinterleave starts as early as possible
                step = I // 4 if tt == 0 else I // 2
                for i0 in range(0, I, step):
                    nc.sync.dma_start(
                        out=xt[:, i0:i0 + step, :], in_=Xp[bs, i0:i0 + step, :]
                    )
                xts[tt] = xt

            load_x(0)
            load_x(1)
            # PD load on the ACT queue: fills ACT's pipeline-warmup idle and
            # keeps the SP queue free for the first two X tiles
            nc.scalar.dma_start(out=pd_sb[:], in_=PDp[:])

            # flat chunk list across tiles; first tile ramps up in 256-row
            # chunks (shorter pipeline fill); last tile tapers off likewise
            # so the final post-lattice PE/evac chain (the tail) is shorter
            all_chunks = []
            for t in range(NT):
                if t == 0:
                    tch = [(0, 256), (256, 256), (512, IH)]
                elif t == NT - 1:
                    tch = [(0, IH), (IH, 256), (IH + 256, 256)]
                else:
                    tch = [(0, IH), (IH, IH)]
                for ic0, ilen in tch:
                    all_chunks.append((t, ic0, ilen))

            # deinterleave on Pool: strided read (i,j)->(j,i); hoisted one
            # chunk ahead so DVE's next-chunk mins never wait on Pool's
            # subtraction backlog
            xjs = {}

            def deint(ci):
                if ci >= len(all_chunks) or ci in xjs:
                    return
                ct, cic0, cilen = all_chunks[ci]
                xj = xj_pool.tile([128, A, cilen], wdt)
                nc.gpsimd.tensor_copy(
                    out=xj[:],
                    in_=xts[ct][:, cic0:cic0 + cilen, :].rearrange(
                        "p i j -> p j i"
                    ),
                )
                xjs[ci] = xj
                if ci + 1 >= len(all_chunks) or all_chunks[ci + 1][0] != ct:
                    xts.pop(ct)  # last chunk of this tile: release xt

            # DVE min-side (pmin trio + merged tmin pair), software-pipelined
            # one chunk ahead of the max-side + Pool subs.
            mins = {}

            def emit_mins(ci):
                if ci >= len(all_chunks) or ci in mins:
                    return
                _, _, cilen = all_chunks[ci]
                cxj = xjs[ci]
                pmin = scr_pool.tile([128, 6, cilen], wdt, tag="pmin")
                tmin = scr_pool.tile([128, 4, cilen], wdt, tag="tmin")
                #  pmin[0:2]=[min01,min23] [2:4]=[min02,min13] [4:6]=[min03,min12]
                nc.vector.tensor_tensor(
                    pmin[:, 0:2], cxj[:, 0::2], cxj[:, 1::2], ALU.min
                )
                nc.vector.tensor_tensor(
                    pmin[:, 2:4], cxj[:, 0:2], cxj[:, 2:4], ALU.min
                )
                nc.vector.tensor_tensor(
                    pmin[:, 4:6], cxj[:, 0:2], cxj[:, 3:1:-1], ALU.min
                )
                # tmin_e = min over X\{e}: tmin[0:2] = min(min23, [x1, x0]);
                # tmin[2:4] = min(min01, [x3, x2])
                nc.vector.tensor_tensor(
                    tmin[:, 0:2],
                    pmin[:, 1:2].broadcast_to([128, 2, cilen]),
                    cxj[:, 1::-1], ALU.min,
                )
                nc.vector.tensor_tensor(
                    tmin[:, 2:4],
                    pmin[:, 0:1].broadcast_to([128, 2, cilen]),
                    cxj[:, 3:1:-1], ALU.min,
                )
                mins[ci] = (pmin, tmin)

            deint(0)
            emit_mins(0)
            for ci, (t, ic0, ilen) in enumerate(all_chunks):
                bsl = slice(t * 128, (t + 1) * 128)
                if ci + 1 < len(all_chunks) and all_chunks[ci + 1][0] != t:
                    load_x(t + 2)
                    load_x(t + 3)
                xj = xjs.pop(ci)
                pmin, tmin = mins.pop(ci)

                pmax = scr_pool.tile([128, 6, ilen], wdt, tag="pmax")
                tmax = scr_pool.tile([128, 4, ilen], wdt, tag="tmax")
                # W grouped: free = (group g, q = s*8 + i_sub)
                w = w_pool.tile([128, ilen // 8, NSLOT * 8], wdt)

                def wslot(s):
                    return w[:, :, s * 8:(s + 1) * 8]

                def grp(ap):
                    return ap.rearrange("p (g e) -> p g e", e=8)

                # DVE max side; pmax[k] = max over complement of the k-th
                # pair so pair-subs align ascending with pmin:
                #  pmax[0:2]=[max23,max01] [2:4]=[max13,max02] [4:6]=[max12,max03]
                nc.vector.tensor_tensor(
                    pmax[:, 0:2], xj[:, 2::-2], xj[:, 3::-2], ALU.max
                )
                nc.vector.tensor_tensor(
                    pmax[:, 2:4], xj[:, 1::-1], xj[:, 3:1:-1], ALU.max
                )
                nc.vector.tensor_tensor(
                    pmax[:, 4:6], xj[:, 1::-1], xj[:, 2:4], ALU.max
                )
                # tmax_e = max over X\{e}: tmax[0:2] = max(max23, [x1, x0]);
                # tmax[2:4] = max(max01, [x3, x2]).  pmax[0]=max23,
                # pmax[1]=max01.
                nc.vector.tensor_tensor(
                    tmax[:, 0:2],
                    pmax[:, 0:1].broadcast_to([128, 2, ilen]),
                    xj[:, 1::-1], ALU.max,
                )
                nc.vector.tensor_tensor(
                    tmax[:, 2:4],
                    pmax[:, 1:2].broadcast_to([128, 2, ilen]),
                    xj[:, 3:1:-1], ALU.max,
                )
                # hoist next chunk's deinterleave to the front of Pool's
                # per-chunk queue (its subs below wait on DVE anyway)
                deint(ci + 1)
                # slot 14 = min4 on DVE
                nc.vector.tensor_tensor(
                    wslot(14), grp(pmin[:, 0]), grp(pmin[:, 1]), ALU.min
                )
                # slot 15 = -min4 on Pool (tensor_scalar mult)
                nc.gpsimd.tensor_scalar(
                    wslot(15), wslot(14), -1.0, None, ALU.mult
                )

                # 14 slot subtractions as 7 double-width ops on Pool (GPSIMD
                # supports subtract; every operand ascending-stride by
                # construction)
                def wpair(s):
                    return w[:, :, s * 8:(s + 2) * 8].rearrange(
                        "p g (s e) -> p s g e", s=2
                    )

                def pl2(tns, a):
                    return tns[:, a:a + 2].rearrange(
                        "p s (g e) -> p s g e", e=8
                    )

                for s0, a_t, a_i, b_t, b_i in (
                    (0, xj, 0, tmax, 0),    # singles {0},{1}
                    (2, xj, 2, tmax, 2),    # singles {2},{3}
                    (4, pmin, 0, pmax, 0),  # pairs {0,1},{2,3}
                    (6, pmin, 2, pmax, 2),  # pairs {0,2},{1,3}
                    (8, pmin, 4, pmax, 4),  # pairs {0,3},{1,2}
                    (10, tmin, 0, xj, 0),   # triples excl 0, excl 1
                    (12, tmin, 2, xj, 2),   # triples excl 2, excl 3
                ):
                    nc.gpsimd.tensor_tensor(
                        wpair(s0), pl2(a_t, a_i), pl2(b_t, b_i),
                        ALU.subtract,
                    )

                # next chunk's DVE min side, ahead of this chunk's relu-evac
                # rounds in the DVE queue
                emit_mins(ci + 1)

                # contraction: per 16 groups of 8 i-rows: 16 transposes fill
                # a 2-bank PSUM tile; one relu-evac (ACT or DVE); 16 matmuls
                # fill a 2-bank psO tile; ACT cast-copies fp32 -> bf16; SP
                # DMAs out.
                for gg in range(0, ilen // 8, 16):
                    it = (t * I + ic0 + 8 * gg) // 128  # global round
                    pT = psT_pool.tile([128, 16, 128], wdt)
                    for u in range(16):
                        nc.tensor.transpose(pT[:, u], w[:, gg + u], ident)
                    lh = lh_pool.tile([128, 16, 128], wdt)
                    # first rounds -> ACT (DVE owns the first lattice); tail
                    # all-DVE (its lattice is done by then); steady state
                    # 2/5 on DVE, phase-tuned -> ~32/64 total on DVE
                    if it < 4:
                        wevac_dve = False
                    elif it >= 52:
                        wevac_dve = True
                    else:
                        wevac_dve = (it + 2) % 5 < 2
                    if wevac_dve:
                        nc.vector.tensor_scalar(
                            lh.rearrange("p a b -> p (a b)"),
                            pT.rearrange("p a b -> p (a b)"),
                            0.0,
                            None,
                            ALU.max,
                        )
                    else:
                        nc.scalar.activation(
                            lh.rearrange("p a b -> p (a b)"),
                            pT.rearrange("p a b -> p (a b)"),
                            AF.Relu,
                        )
                    pO = psO_pool.tile([128, 16, 64], f32)
                    for u in range(16):
                        g = gg + u          # local group in this chunk
                        gG = ic0 // 8 + g   # global group
                        nc.tensor.matmul(
                            pO[:, u],
                            lhsT=lh[:, u],
                            rhs=pd_sb[:, gG * 64:(gG + 1) * 64],
                            start=True,
                            stop=True,
                        )
                    ot = ot_pool.tile([128, 16, 64], wdt)
                    i0 = ic0 + gg * 8
                    if it == 63:
                        # final round: split the cast-copy + OUT DMA in half
                        # so the last DMA's fixed DGE latency overlaps the
                        # second half's copy
                        for h0 in (0, 8):
                            nc.scalar.activation(
                                ot[:, h0:h0 + 8].rearrange("p a b -> p (a b)"),
                                pO[:, h0:h0 + 8].rearrange("p a b -> p (a b)"),
                                AF.Copy,
                            )
                            nc.sync.dma_start(
                                out=OUTp[bsl, i0 + h0 * 8:i0 + h0 * 8 + 64, :],
                                in_=ot[:, h0:h0 + 8].rearrange(
                                    "p g (i o) -> p (g i) o", o=8
                                ),
                            )
                    else:
                        nc.scalar.activation(
                            ot.rearrange("p a b -> p (a b)"),
                            pO.rearrange("p a b -> p (a b)"),
                            AF.Copy,
                        )
                        nc.sync.dma_start(
                            out=OUTp[bsl, i0:i0 + 128, :],
                            in_=ot.rearrange("p g (i o) -> p (g i) o", o=8),
                        )
    if not nc.is_finalized():
        nc.finalize()
    return nc


def _get_nc():
    if "nc" not in _CACHE:
        _CACHE["nc"] = _build_bass()
    return _CACHE["nc"]


def kernel(X: np.ndarray, params: np.ndarray) -> np.ndarray:
    from concourse.bass_utils import run_bass_kernel_spmd

    X = np.asarray(X, dtype=np.float32).astype(ml_dtypes.bfloat16)
    params = np.asarray(params, dtype=np.float32)
    PD = _build_pd(params)

    nc = _get_nc()
    in_maps = [
        {"X": X[c * BC:(c + 1) * BC], "PD": PD} for c in range(NCORES)
    ]
    res = run_bass_kernel_spmd(nc, in_maps, list(range(NCORES)))
    out = np.concatenate(
        [np.asarray(res.results[c]["OUT"]) for c in range(NCORES)], axis=0
    )
    return out.astype(np.float32)


# revision 86
# speedup vs baseline: 1.0013x; 1.0013x over previous
"""Trainium2 Bass kernel for nn_BatchHighOrderActivation.

Math: out[b,i,o] = sum_k coef_k * params[i, idx_k, o]  (sorted-diff coefs,
reverse-cumsum subset masks).  Rewritten gather-free as

    out[b,i,:] = sum_{m=1..15} w_m[b,i] * params[i, m, :]
    w_m = relu( min_{j in m} X_j  -  max_{j not in m} X_j )   (m != 15)
    w_15 = min4 = relu(min4) - relu(-min4)  (split across two relu slots)

Per core (batch-sharded 8 ways, 1024 batch rows each), per 128-row b-tile:
  1. SP HWDGE loads X tile (host pre-casts X to bf16).
  2. Pool: deinterleave -> 4 planes X_j (strided tensor_copy).
  3. DVE: the full min/max lattice (pmin/pmax/tmin/tmax/s14) -- GPSIMD has
     no tensor-tensor min/max ucode, so these must stay on DVE (2x bf16).
  4. Pool: the 14 slot subtractions (GPSIMD supports subtract; all operand
     access patterns ascending -- GPSIMD also rejects negative-stride and
     broadcast APs) + the s15 negation.
  5. PE:  transpose W group-tiles ([128b x 128q]) -> PSUM bf16.
  6. ACT/DVE: relu-evacuate PSUM -> lhsT tiles [128q, 128b] bf16.
     (GPSIMD cannot touch PSUM.)
  7. PE:  matmul lhsT.T @ PD[g] (PD = block-diag P, K=q) -> PSUM fp32.
  8. ACT: cast-copy PSUM fp32 -> SBUF bf16.
  9. SP:  DMA out bf16 (host upcasts to fp32 after gather).

Engine busy budget per core (v1 cost model): DVE ~136us (lattice + relu
share), ACT ~136us (relu share + out-evacs), Pool ~130us (deint + subs),
SP ~82us DMA, PE ~82us.
"""

import sys

for _p in ("/opt/trn_rl_repo", "/root/.axon_site/_ro/trn_rl_repo"):
    if _p not in sys.path:
        sys.path.append(_p)

import numpy as np
import ml_dtypes

B, I, A, O = 8192, 1024, 4, 8
NCORES = 8
BC = B // NCORES          # batch rows per core
NG = I // 8               # 128 groups of 8 i-rows
NSLOT = 16

# slot order chosen so merged double-width subs write adjacent slots:
# s0..3 singles {0}{1}{2}{3}; s4..9 pair-masks in PAIRS order; s10..13
# triples ordered by excluded coordinate; s14/15 = +/- full-set (mask 15)
SLOT_MASKS = [1, 2, 4, 8, 3, 12, 5, 10, 9, 6, 14, 13, 11, 7]

_CACHE = {}


def _build_pd(params: np.ndarray) -> np.ndarray:
    """Block-diagonal P table: PD[q = s*8 + i_sub, g, n = i_sub*8 + o]."""
    Pt = np.empty((I, NSLOT, O), np.float32)
    for s, m in enumerate(SLOT_MASKS):
        Pt[:, s, :] = params[:, m, :]
    Pt[:, 14, :] = params[:, 15, :]
    Pt[:, 15, :] = -params[:, 15, :]

    PD = np.zeros((128, NG, 64), np.float32)
    for s in range(NSLOT):
        for isub in range(8):
            PD[s * 8 + isub, :, isub * 8:(isub + 1) * 8] = Pt[
                np.arange(NG) * 8 + isub, s, :
            ]
    return PD.reshape(128, NG * 64).astype(ml_dtypes.bfloat16)


def _build_bass():
    import concourse.bass as bass
    import concourse.mybir as mybir
    import concourse.tile as tile
    from concourse import bacc
    from concourse.masks import make_identity

    f32 = mybir.dt.float32
    wdt = mybir.dt.bfloat16

    nc = bacc.Bacc(None)
    Xp = nc.declare_dram_parameter("X", [BC, I, A], wdt, isOutput=False)
    PDp = nc.declare_dram_parameter("PD", [128, NG * 64], wdt, isOutput=False)
    OUTp = nc.declare_dram_parameter("OUT", [BC, I, O], wdt, isOutput=True)

    AF = mybir.ActivationFunctionType
    ALU = mybir.AluOpType

    IH = I // 2    # i-half extent per lattice pass

    with tile.TileContext(nc) as tc:
        with (
            tc.tile_pool(name="consts", bufs=1) as consts,
            tc.tile_pool(name="xin", bufs=5) as xin_pool,
            tc.tile_pool(name="xj", bufs=3) as xj_pool,
            tc.tile_pool(name="scr", bufs=2) as scr_pool,
            tc.tile_pool(name="w", bufs=3) as w_pool,
            tc.tile_pool(name="lh", bufs=4) as lh_pool,
            tc.tile_pool(name="ot", bufs=5) as ot_pool,
            tc.tile_pool(name="psT", bufs=2, space="PSUM") as psT_pool,
            tc.tile_pool(name="psO", bufs=2, space="PSUM") as psO_pool,
        ):
            ident = consts.tile([128, 128], wdt)
            make_identity(nc, ident)
            pd_sb = consts.tile([128, NG * 64], wdt)

            NT = BC // 128
            # X tiles prefetched with lookahead >= 2 so the loads run ahead
            # of the same-tile OUT DMAs in the SP queue's program order
            xts = {}

            def load_x(tt):
                if tt >= NT or tt in xts:
                    return
                bs = slice(tt * 128, (tt + 1) * 128)
                xt = xin_pool.tile([128, I, A], wdt)
                # tile 0 loads in quarters so the first 256-row chunk's
                # deinterleave starts as early as possible
                step = I // 4 if tt == 0 else I // 2
                for i0 in range(0, I, step):
                    nc.sync.dma_start(
                        out=xt[:, i0:i0 + step, :], in_=Xp[bs, i0:i0 + step, :]
                    )
                xts[tt] = xt

            load_x(0)
            load_x(1)
            # PD load on the ACT queue: fills ACT's pipeline-warmup idle and
            # keeps the SP queue free for the first two X tiles
            nc.scalar.dma_start(out=pd_sb[:], in_=PDp[:])

            # flat chunk list across tiles; first tile ramps up in 256-row
            # chunks (shorter pipeline fill); last tile tapers off likewise
            # so the final post-lattice PE/evac chain (the tail) is shorter
            all_chunks = []
            for t in range(NT):
                if t == 0:
                    tch = [(0, 256), (256, 256), (512, IH)]
                elif t == NT - 1:
                    tch = [(0, IH), (IH, 256), (IH + 256, 256)]
                else:
                    tch = [(0, IH), (IH, IH)]
                for ic0, ilen in tch:
                    all_chunks.append((t, ic0, ilen))

            # deinterleave on Pool: strided read (i,j)->(j,i); hoisted one
            # chunk ahead so DVE's next-chunk mins never wait on Pool's
            # subtraction backlog
            xjs = {}

            def deint(ci):
                if ci >= len(all_chunks) or ci in xjs:
                    return
                ct, cic0, cilen = all_chunks[ci]
                xj = xj_pool.tile([128, A, cilen], wdt)
                nc.gpsimd.tensor_copy(
                    out=xj[:],
                    in_=xts[ct][:, cic0:cic0 + cilen, :].rearrange(
                        "p i j -> p j i"
                    ),
                )
                xjs[ci] = xj
                if ci + 1 >= len(all_chunks) or all_chunks[ci + 1][0] != ct:
                    xts.pop(ct)  # last chunk of this tile: release xt

            # DVE min-side (pmin trio + merged tmin pair), software-pipelined
            # one chunk ahead of the max-side + Pool subs.
            mins = {}

            def emit_mins(ci):
                if ci >= len(all_chunks) or ci in mins:
                    return
                _, _, cilen = all_chunks[ci]
                cxj = xjs[ci]
                pmin = scr_pool.tile([128, 6, cilen], wdt, tag="pmin")
                tmin = scr_pool.tile([128, 4, cilen], wdt, tag="tmin")
                #  pmin[0:2]=[min01,min23] [2:4]=[min02,min13] [4:6]=[min03,min12]
                nc.vector.tensor_tensor(
                    pmin[:, 0:2], cxj[:, 0::2], cxj[:, 1::2], ALU.min
                )
                nc.vector.tensor_tensor(
                    pmin[:, 2:4], cxj[:, 0:2], cxj[:, 2:4], ALU.min
                )
                nc.vector.tensor_tensor(
                    pmin[:, 4:6], cxj[:, 0:2], cxj[:, 3:1:-1], ALU.min
                )
                # tmin_e = min over X\{e}: tmin[0:2] = min(min23, [x1, x0]);
                # tmin[2:4] = min(min01, [x3, x2])
                nc.vector.tensor_tensor(
                    tmin[:, 0:2],
                    pmin[:, 1:2].broadcast_to([128, 2, cilen]),
                    cxj[:, 1::-1], ALU.min,
                )
                nc.vector.tensor_tensor(
                    tmin[:, 2:4],
                    pmin[:, 0:1].broadcast_to([128, 2, cilen]),
                    cxj[:, 3:1:-1], ALU.min,
                )
                mins[ci] = (pmin, tmin)

            deint(0)
            emit_mins(0)
            for ci, (t, ic0, ilen) in enumerate(all_chunks):
                bsl = slice(t * 128, (t + 1) * 128)
                if ci + 1 < len(all_chunks) and all_chunks[ci + 1][0] != t:
                    load_x(t + 2)
                    load_x(t + 3)
                xj = xjs.pop(ci)
                pmin, tmin = mins.pop(ci)

                pmax = scr_pool.tile([128, 6, ilen], wdt, tag="pmax")
                tmax = scr_pool.tile([128, 4, ilen], wdt, tag="tmax")
                # W grouped: free = (group g, q = s*8 + i_sub)
                w = w_pool.tile([128, ilen // 8, NSLOT * 8], wdt)

                def wslot(s):
                    return w[:, :, s * 8:(s + 1) * 8]

                def grp(ap):
                    return ap.rearrange("p (g e) -> p g e", e=8)

                # DVE max side; pmax[k] = max over complement of the k-th
                # pair so pair-subs align ascending with pmin:
                #  pmax[0:2]=[max23,max01] [2:4]=[max13,max02] [4:6]=[max12,max03]
                nc.vector.tensor_tensor(
                    pmax[:, 0:2], xj[:, 2::-2], xj[:, 3::-2], ALU.max
                )
                nc.vector.tensor_tensor(
                    pmax[:, 2:4], xj[:, 1::-1], xj[:, 3:1:-1], ALU.max
                )
                nc.vector.tensor_tensor(
                    pmax[:, 4:6], xj[:, 1::-1], xj[:, 2:4], ALU.max
                )
                # tmax_e = max over X\{e}: tmax[0:2] = max(max23, [x1, x0]);
                # tmax[2:4] = max(max01, [x3, x2]).  pmax[0]=max23,
                # pmax[1]=max01.
                nc.vector.tensor_tensor(
                    tmax[:, 0:2],
                    pmax[:, 0:1].broadcast_to([128, 2, ilen]),
                    xj[:, 1::-1], ALU.max,
                )
                nc.vector.tensor_tensor(
                    tmax[:, 2:4],
                    pmax[:, 1:2].broadcast_to([128, 2, ilen]),
                    xj[:, 3:1:-1], ALU.max,
                )
                # hoist next chunk's deinterleave to the front of Pool's
                # per-chunk queue (its subs below wait on DVE anyway)
                deint(ci + 1)
                # slot 14 = min4 on DVE
                nc.vector.tensor_tensor(
                    wslot(14), grp(pmin[:, 0]), grp(pmin[:, 1]), ALU.min
                )
                # slot 15 = -min4 on Pool (tensor_scalar mult)
                nc.gpsimd.tensor_scalar(
                    wslot(15), wslot(14), -1.0, None, ALU.mult
                )

                # 14 slot subtractions as 7 double-width ops on Pool (GPSIMD
                # supports subtract; every operand ascending-stride by
                # construction)
                def wpair(s):
                    return w[:, :, s * 8:(s + 2) * 8].rearrange(
                        "p g (s e) -> p s g e", s=2
                    )

                def pl2(tns, a):
                    return tns[:, a:a + 2].rearrange(
                        "p s (g e) -> p s g e", e=8
                    )

                for s0, a_t, a_i, b_t, b_i in (
                    (0, xj, 0, tmax, 0),    # singles {0},{1}
                    (2, xj, 2, tmax, 2),    # singles {2},{3}
                    (4, pmin, 0, pmax, 0),  # pairs {0,1},{2,3}
                    (6, pmin, 2, pmax, 2),  # pairs {0,2},{1,3}
                    (8, pmin, 4, pmax, 4),  # pairs {0,3},{1,2}
                    (10, tmin, 0, xj, 0),   # triples excl 0, excl 1
                    (12, tmin, 2, xj, 2),   # triples excl 2, excl 3
                ):
                    nc.gpsimd.tensor_tensor(
                        wpair(s0), pl2(a_t, a_i), pl2(b_t, b_i),
                        ALU.subtract,
                    )

                # next chunk's DVE min side, ahead of this chunk's relu-evac
                # rounds in the DVE queue
                emit_mins(ci + 1)

                # contraction: per 16 groups of 8 i-rows: 16 transposes fill
                # a 2-bank PSUM tile; one relu-evac (ACT or DVE); 16 matmuls
                # fill a 2-bank psO tile; ACT cast-copies fp32 -> bf16; SP
                # DMAs out.
                for gg in range(0, ilen // 8, 16):
                    it = (t * I + ic0 + 8 * gg) // 128  # global round
                    pT = psT_pool.tile([128, 16, 128], wdt)
                    for u in range(16):
                        nc.tensor.transpose(pT[:, u], w[:, gg + u], ident)
                    lh = lh_pool.tile([128, 16, 128], wdt)
                    # first rounds -> ACT (DVE owns the first lattice); tail
                    # all-DVE (its lattice is done by then); steady state
                    # 2/5 on DVE, phase-tuned -> ~32/64 total on DVE
                    if it < 4:
                        wevac_dve = False
                    elif it >= 52:
                        wevac_dve = True
                    else:
                        wevac_dve = (it + 1) % 5 < 2
                    if wevac_dve:
                        nc.vector.tensor_scalar(
                            lh.rearrange("p a b -> p (a b)"),
                            pT.rearrange("p a b -> p (a b)"),
                            0.0,
                            None,
                            ALU.max,
                        )
                    else:
                        nc.scalar.activation(
                            lh.rearrange("p a b -> p (a b)"),
                            pT.rearrange("p a b -> p (a b)"),
                            AF.Relu,
                        )
                    pO = psO_pool.tile([128, 16, 64], f32)
                    for u in range(16):
                        g = gg + u          # local group in this chunk
                        gG = ic0 // 8 + g   # global group
                        nc.tensor.matmul(
                            pO[:, u],
                            lhsT=lh[:, u],
                            rhs=pd_sb[:, gG * 64:(gG + 1) * 64],
                            start=True,
                            stop=True,
                        )
                    ot = ot_pool.tile([128, 16, 64], wdt)
                    i0 = ic0 + gg * 8
                    if it == 63:
                        # final round: split the cast-copy + OUT DMA in half
                        # so the last DMA's fixed DGE latency overlaps the
                        # second half's copy
                        for h0 in (0, 8):
                            nc.scalar.activation(
                                ot[:, h0:h0 + 8].rearrange("p a b -> p (a b)"),
                                pO[:, h0:h0 + 8].rearrange("p a b -> p (a b)"),
                                AF.Copy,
                            )
                            nc.sync.dma_start(
                                out=OUTp[bsl, i0 + h0 * 8:i0 + h0 * 8 + 64, :],
                                in_=ot[:, h0:h0 + 8].rearrange(
                                    "p g (i o) -> p (g i) o", o=8
                                ),
                            )
                    else:
                        nc.scalar.activation(
                            ot.rearrange("p a b -> p (a b)"),
                            pO.rearrange("p a b -> p (a b)"),
                            AF.Copy,
                        )
                        nc.sync.dma_start(
                            out=OUTp[bsl, i0:i0 + 128, :],
                            in_=ot.rearrange("p g (i o) -> p (g i) o", o=8),
                        )
    if not nc.is_finalized():
        nc.finalize()
    return nc


def _get_nc():
    if "nc" not in _CACHE:
        _CACHE["nc"] = _build_bass()
    return _CACHE["nc"]


def kernel(X: np.ndarray, params: np.ndarray) -> np.ndarray:
    from concourse.bass_utils import run_bass_kernel_spmd

    X = np.asarray(X, dtype=np.float32).astype(ml_dtypes.bfloat16)
    params = np.asarray(params, dtype=np.float32)
    PD = _build_pd(params)

    nc = _get_nc()
    in_maps = [
        {"X": X[c * BC:(c + 1) * BC], "PD": PD} for c in range(NCORES)
    ]
    res = run_bass_kernel_spmd(nc, in_maps, list(range(NCORES)))
    out = np.concatenate(
        [np.asarray(res.results[c]["OUT"]) for c in range(NCORES)], axis=0
    )
    return out.astype(np.float32)
